# revision 18
# baseline (speedup 1.0000x reference)
"""Trainium2 Bass kernel for NeuralCTLSTM cell (B=65536, H=256, 7 gates).

Data-parallel over 8 NeuronCores (8192 batch rows each). Per core, per
128-row block:
  gates = h @ Wp^T + bp  (7 gates in one PSUM tile [128,1792], bf16 matmuls,
                          K=256 in 2 chunks + a K=1 ones-row for the bias)
  gate columns: [f, o, fbar, i, ibar, dneg, z] where the dneg slot holds the
  NEGATED decay-gate weights, so sigmoid drains 6 contiguous gates at once:
    sigma(-g_d) -> lns = ln(sigma(-g_d)) = -softplus(g_d) = -decay
    decay = -lns ; e = exp(-dt*decay) = exp(dt*lns)
  h^T needed by the PE comes straight from DRAM via DMA-transpose (bf16).
  c_after = cbar + (c-cbar)*e ; outputs og, hn, cn, cbn, dec (all bf16,
  host converts to fp32).

ACT table phasing: supergroups of 16 blocks; sigmoid/tanh (sigmoid set) for
the supergroup, then ln/exp (natural_log_exp set). tanh(c_after) of
supergroup g runs at the start of the sigmoid phase of g+1. Zero-valued
[P,1] tokens (passed as activation bias) enforce the phase edges.
"""

import os
import sys

sys.path.insert(0, "/opt/trn_rl_repo")

from contextlib import ExitStack

import numpy as np

NCORES = 8
B, H, G = 65536, 256, 7
P = 128
BL = B // NCORES           # rows per core
NBLK = BL // P             # 64 row-blocks per core
ST = 4                     # blocks per group (DMA/DVE batch)
NGRP = NBLK // ST          # 16 groups per core
GPS = 4                    # groups per supergroup
NSG = NGRP // GPS          # 4 supergroups
GH = G * H                 # 1792
SIGW = 6 * H               # 1536: f,o,fbar,i,ibar,dneg drained by one sigmoid

# gate column slots -> reference gate index (reference order:
# 0=input 1=forget 2=output 3=ibar 4=fbar 5=z 6=decay). Slot 5 (dneg) is
# negated on the host.
GATE_PERM = [1, 2, 4, 0, 3, 6, 5]

_NC = None


def build_nc():
    from concourse import bacc, mybir
    from concourse.tile import TileContext

    F32 = mybir.dt.float32
    BF16 = mybir.dt.bfloat16
    AF = mybir.ActivationFunctionType

    nc = bacc.Bacc("TRN2", target_bir_lowering=False, debug=False)

    # hT: host-pre-transposed h, [k-chunk, feature, batch-row] per core.
    # c/cbar: partition-major [row%128, block*H + col] so per-partition DMA
    # segments are contiguous (2 KB per block instead of 512 B lines).
    hT_d = nc.dram_tensor("hT", [2, P, BL], BF16, kind="ExternalInput")
    c_d = nc.dram_tensor("c", [P, NBLK * H], BF16, kind="ExternalInput")
    cb_d = nc.dram_tensor("cbar", [P, NBLK * H], BF16, kind="ExternalInput")
    dt_d = nc.dram_tensor("dts", [P, NBLK], F32, kind="ExternalInput")
    wt_d = nc.dram_tensor("wt", [2, P, GH], BF16, kind="ExternalInput")
    bia_d = nc.dram_tensor("bias", [1, GH], BF16, kind="ExternalInput")

    og_d = nc.dram_tensor("og", [BL, H], BF16, kind="ExternalOutput")
    hn_d = nc.dram_tensor("hn", [BL, H], BF16, kind="ExternalOutput")
    cn_d = nc.dram_tensor("cn", [BL, H], BF16, kind="ExternalOutput")
    cbn_d = nc.dram_tensor("cbn", [BL, H], BF16, kind="ExternalOutput")
    dec_d = nc.dram_tensor("dec", [BL, H], BF16, kind="ExternalOutput")

    og_r = og_d.rearrange("(n p) d -> n p d", p=P)
    hn_r = hn_d.rearrange("(n p) d -> n p d", p=P)
    cn_r = cn_d.rearrange("(n p) d -> n p d", p=P)
    cbn_r = cbn_d.rearrange("(n p) d -> n p d", p=P)
    dec_r = dec_d.rearrange("(n p) d -> n p d", p=P)

    def dsl(r, b0, n):  # dram slice of n row-blocks as [P, n, H]
        return r[b0 : b0 + n].rearrange("n p d -> p n d")

    MU = mybir.AluOpType.mult

    with TileContext(nc) as tc, ExitStack() as ctx:
        pool = lambda name, bufs, **kw: ctx.enter_context(
            tc.tile_pool(name=name, bufs=bufs, **kw)
        )
        const = pool("const", 1)
        hTp = pool("hTp", 2)
        cp = pool("cp", 2)
        cbp = pool("cbp", 2)
        Sp = pool("Sp", 6)
        Zp = pool("Zp", 6)
        lnsp = pool("lnsp", 4)
        einp = pool("einp", 2)
        ep = pool("ep", 3)
        cap = pool("cap", 6)
        tcap = pool("tcap", 2)
        tmpp = pool("tmpp", 2)
        hnp = pool("hnp", 2)
        cnp = pool("cnp", 2)
        cbnp = pool("cbnp", 2)
        ndecp = pool("ndecp", 2)
        tokp = pool("tokp", 4)
        pgp = pool("pgp", 2, space="PSUM")

        wt_sb = const.tile([P, 2, GH], BF16)
        nc.sync.dma_start(wt_sb[:], wt_d.rearrange("c k g -> k c g"))
        bia_sb = const.tile([1, GH], BF16)
        nc.sync.dma_start(bia_sb[:], bia_d[:, :])
        dts_sb = const.tile([P, NBLK], F32)
        nc.sync.dma_start(dts_sb[:], dt_d[:, :])
        ones_sb = const.tile([1, P], BF16)
        nc.vector.memset(ones_sb[:], 1.0)

        BANKS = [(0, 512), (512, 1024), (1024, 1536), (1536, 1792)]

        tokA = None        # gates SIG-phase ops of SG g after NLE ops of g-1
        prev = []          # [(S, ca, grp), ...] of previous supergroup

        SGB = GPS * ST  # blocks per supergroup (16)
        for g in range(NSG):
            # ---------------- SIG phase: matmuls + sigmoid/tanh drains ------
            hT = hTp.tile([P, 2, SGB * P], BF16)
            nc.sync.dma_start(
                hT[:],
                hT_d[:, :, g * SGB * P : (g + 1) * SGB * P].rearrange(
                    "c k r -> k c r"
                ),
            )
            c_sg = cp.tile([P, SGB, H], BF16)
            cb_sg = cbp.tile([P, SGB, H], BF16)
            nc.sync.dma_start(
                c_sg[:],
                c_d[:, g * SGB * H : (g + 1) * SGB * H].rearrange(
                    "k (n d) -> k n d", d=H
                ),
            )
            nc.sync.dma_start(
                cb_sg[:],
                cb_d[:, g * SGB * H : (g + 1) * SGB * H].rearrange(
                    "k (n d) -> k n d", d=H
                ),
            )
            sg_S = []
            sg_Z = []
            tca_last = None
            for t in range(GPS):
                S = Sp.tile([P, ST, SIGW], BF16)
                Z = Zp.tile([P, ST, H], BF16)
                for j in range(ST):
                    jj = t * ST + j
                    Gp = pgp.tile([P, 2048], mybir.dt.float32)
                    for lo, hi in BANKS:
                        nc.tensor.matmul(
                            Gp[:, lo:hi], ones_sb[:], bia_sb[:, lo:hi],
                            start=True, stop=False,
                        )
                    for ck in range(2):
                        lhsT = hT[:, ck, jj * P : (jj + 1) * P]
                        for lo, hi in BANKS:
                            nc.tensor.matmul(
                                Gp[:, lo:hi], lhsT, wt_sb[:, ck, lo:hi],
                                start=False, stop=(ck == 1),
                            )
                    if tokA is not None:
                        nc.scalar.activation(
                            S[:, j, :], Gp[:, 0:SIGW], AF.Sigmoid, bias=tokA[:]
                        )
                        nc.scalar.activation(
                            Z[:, j, :], Gp[:, SIGW:GH], AF.Tanh, bias=tokA[:]
                        )
                    else:
                        nc.scalar.activation(S[:, j, :], Gp[:, 0:SIGW], AF.Sigmoid)
                        nc.scalar.activation(Z[:, j, :], Gp[:, SIGW:GH], AF.Tanh)
                sg_S.append(S)
                sg_Z.append(Z)
                # previous supergroup, same group index: tanh(c_after) + hn
                if prev:
                    pS, pca, pgrp = prev[t]
                    tca = tcap.tile([P, ST, H], BF16)
                    if tokA is not None:
                        nc.scalar.activation(tca[:], pca[:], AF.Tanh, bias=tokA[:])
                    else:
                        nc.scalar.activation(tca[:], pca[:], AF.Tanh)
                    hn = hnp.tile([P, ST, H], BF16)
                    nc.vector.tensor_mul(hn[:], pS[:, :, H : 2 * H], tca[:])
                    nc.sync.dma_start(dsl(hn_r, pgrp * ST, ST), hn[:])
                    tca_last = tca

            # tokB: all NLE-set ACT ops of this SG wait on the SIG-set ops
            tokB = tokp.tile([P, 1], mybir.dt.float32)
            if tca_last is not None:
                nc.vector.scalar_tensor_tensor(
                    tokB[:], sg_Z[-1][:, ST - 1, 0:1], 0.0,
                    tca_last[:, ST - 1, 0:1], MU, MU,
                )
            else:
                nc.vector.tensor_scalar_mul(tokB[:], sg_Z[-1][:, ST - 1, 0:1], 0.0)

            # ---------------- NLE phase: ln/exp + elementwise + outputs -----
            # All Ln ops first, then all Exp ops: one natural_log table load
            # and one exp_and_others load per supergroup instead of 2 each.
            sg_lns = []
            for t in range(GPS):
                lns = lnsp.tile([P, ST, H], BF16)
                nc.scalar.activation(
                    lns[:], sg_S[t][:, :, 5 * H : 6 * H], AF.Ln, bias=tokB[:]
                )
                sg_lns.append(lns)
            sg_e = []
            for t in range(GPS):
                b0 = (g * GPS + t) * ST
                lns = sg_lns[t]
                ndec = ndecp.tile([P, ST, H], BF16)
                nc.vector.tensor_scalar_mul(ndec[:], lns[:], -1.0)
                nc.sync.dma_start(dsl(dec_r, b0, ST), ndec[:])
                ein = einp.tile([P, ST, H], BF16)
                for j in range(ST):
                    blk = b0 + j
                    nc.vector.tensor_scalar_mul(
                        ein[:, j, :], lns[:, j, :], dts_sb[:, blk : blk + 1]
                    )
                e = ep.tile([P, ST, H], BF16)
                nc.scalar.activation(e[:], ein[:], AF.Exp, bias=tokB[:])
                sg_e.append(e)
            prev_new = []
            for t in range(GPS):
                grp = g * GPS + t
                b0 = grp * ST
                S, Z, e = sg_S[t], sg_Z[t], sg_e[t]
                c_t = c_sg[:, t * ST : (t + 1) * ST, :]
                cb_t = cb_sg[:, t * ST : (t + 1) * ST, :]
                # elementwise chain (DVE, bf16 2x); two reused scratch tiles
                tA = tmpp.tile([P, ST, H], BF16)
                tB = tmpp.tile([P, ST, H], BF16)
                nc.vector.tensor_sub(tA[:], c_t, cb_t)          # d = c - cb
                nc.vector.tensor_mul(tB[:], tA[:], e[:])        # m = d * e
                ca = cap.tile([P, ST, H], BF16)
                nc.vector.tensor_add(ca[:], tB[:], cb_t)        # ca = m + cb
                nc.vector.tensor_mul(tA[:], S[:, :, 3 * H : 4 * H], Z[:])  # iz
                nc.vector.tensor_mul(tB[:], S[:, :, 0:H], ca[:])           # f*ca
                cn = cnp.tile([P, ST, H], BF16)
                nc.vector.tensor_add(cn[:], tB[:], tA[:])
                nc.sync.dma_start(dsl(cn_r, b0, ST), cn[:])
                nc.vector.tensor_mul(tA[:], S[:, :, 2 * H : 3 * H], cb_t)  # fbar*cb
                nc.vector.tensor_mul(tB[:], S[:, :, 4 * H : 5 * H], Z[:])  # ibar*z
                cbn = cbnp.tile([P, ST, H], BF16)
                nc.vector.tensor_add(cbn[:], tA[:], tB[:])
                nc.sync.dma_start(dsl(cbn_r, b0, ST), cbn[:])
                nc.sync.dma_start(dsl(og_r, b0, ST), S[:, :, H : 2 * H])
                prev_new.append((S, ca, grp))

            prev = prev_new
            tokA = tokp.tile([P, 1], mybir.dt.float32)
            nc.vector.tensor_scalar_mul(tokA[:], sg_e[-1][:, ST - 1, 0:1], 0.0)

        # epilogue: hn for the final supergroup
        for t in range(GPS):
            pS, pca, pgrp = prev[t]
            tca = tcap.tile([P, ST, H], BF16)
            nc.scalar.activation(tca[:], pca[:], AF.Tanh, bias=tokA[:])
            hn = hnp.tile([P, ST, H], BF16)
            nc.vector.tensor_mul(hn[:], pS[:, :, H : 2 * H], tca[:])
            nc.sync.dma_start(dsl(hn_r, pgrp * ST, ST), hn[:])

    nc.compile()
    return nc


def prep_weights(W, b):
    """W [7,256,256] fp32, b [7,256] fp32 -> wt [2,128,1792] bf16, bias [1,1792] bf16."""
    import ml_dtypes

    Wp = np.ascontiguousarray(W[GATE_PERM]).astype(np.float32).copy()
    bp = np.ascontiguousarray(b[GATE_PERM]).astype(np.float32).copy()
    Wp[5] = -Wp[5]  # dneg slot: sigma(-g_d)
    bp[5] = -bp[5]
    # wt[ck, k, g*H+o] = Wp[g, o, ck*128+k]
    wt = np.transpose(Wp, (2, 0, 1)).reshape(H, GH)
    wt = np.ascontiguousarray(wt.reshape(2, P, GH)).astype(ml_dtypes.bfloat16)
    bia = bp.reshape(1, GH).astype(ml_dtypes.bfloat16)
    return wt, bia


_RUNNER = None


def _make_runner(nc):
    """Cached shard_map-jitted executor for nc across 8 cores."""
    import jax
    from jax.experimental.shard_map import shard_map
    from jax.sharding import Mesh, PartitionSpec

    from concourse import bass2jax, mybir

    bass2jax.install_neuronx_cc_hook()
    p = bass2jax._bass_exec_p

    part_name = nc.partition_id_tensor.name if nc.partition_id_tensor else None
    in_names, out_names, out_avals = [], [], []
    for alloc in nc.m.functions[0].allocations:
        if not isinstance(alloc, mybir.MemoryLocationSet):
            continue
        name = alloc.memorylocations[0].name
        if alloc.kind == "ExternalInput":
            if name != part_name:
                in_names.append(name)
        elif alloc.kind == "ExternalOutput":
            out_names.append(name)
            out_avals.append(
                jax.core.ShapedArray(tuple(alloc.tensor_shape), mybir.dt.np(alloc.dtype))
            )
    n_params = len(in_names)
    all_in = in_names + out_names
    if part_name is not None:
        all_in = all_in + [part_name]

    def _body(*args):
        operands = list(args)
        if part_name is not None:
            operands.append(bass2jax.partition_id_tensor())
        return tuple(
            p.bind(
                *operands,
                out_avals=tuple(out_avals),
                in_names=tuple(all_in),
                out_names=tuple(out_names),
                lowering_input_output_aliases=(),
                sim_require_finite=True,
                sim_require_nnan=True,
                nc=nc,
            )
        )

    devices = jax.devices()[:NCORES]
    mesh = Mesh(np.asarray(devices), ("core",))
    nin = n_params + len(out_names)
    sharded = jax.jit(
        shard_map(
            _body,
            mesh=mesh,
            in_specs=(PartitionSpec("core"),) * nin,
            out_specs=(PartitionSpec("core"),) * len(out_names),
            check_rep=False,
        ),
        donate_argnums=tuple(range(n_params, nin)),
        keep_unused=True,
    )
    return sharded, in_names, out_names, out_avals, mesh


def get_runner():
    global _NC, _RUNNER
    if _RUNNER is None:
        if _NC is None:
            _NC = build_nc()
        _RUNNER = _make_runner(_NC)
    return _RUNNER


def make_concat_inputs(inter_times, h_ti, c_ti, cbar, W, b):
    """Global (8*shape[0], ...) arrays keyed by dram tensor name."""
    import ml_dtypes

    BF = ml_dtypes.bfloat16
    inter_times = np.asarray(inter_times, dtype=np.float32)
    wt, bia = prep_weights(np.asarray(W, np.float32), np.asarray(b, np.float32))
    dts = np.ascontiguousarray(
        inter_times.reshape(NCORES, NBLK, P).transpose(0, 2, 1)
    ).reshape(NCORES * P, NBLK)
    # hT: [core, k-chunk, feature, row] -> concat [(core chunk), P, BL]
    h_bf = np.asarray(h_ti).astype(BF)
    hT = np.ascontiguousarray(
        h_bf.reshape(NCORES, BL, 2, P).transpose(0, 2, 3, 1)
    ).reshape(NCORES * 2, P, BL)
    # c/cbar: partition-major [core, row%128, block, col] -> [(core P), NBLK*H]
    def pmaj(x):
        xb = np.asarray(x).astype(BF)
        return np.ascontiguousarray(
            xb.reshape(NCORES, NBLK, P, H).transpose(0, 2, 1, 3)
        ).reshape(NCORES * P, NBLK * H)

    return {
        "hT": hT,
        "c": pmaj(c_ti),
        "cbar": pmaj(cbar),
        "dts": dts,
        "wt": np.ascontiguousarray(np.broadcast_to(wt, (NCORES,) + wt.shape)).reshape(
            NCORES * wt.shape[0], *wt.shape[1:]
        ),
        "bias": np.ascontiguousarray(
            np.broadcast_to(bia, (NCORES,) + bia.shape)
        ).reshape(NCORES * bia.shape[0], *bia.shape[1:]),
    }


def device_zeros(out_avals, mesh):
    import jax.numpy as jnp
    from jax.sharding import NamedSharding, PartitionSpec

    sh = NamedSharding(mesh, PartitionSpec("core"))
    return [
        jnp.zeros((NCORES * a.shape[0], *a.shape[1:]), a.dtype, device=sh)
        for a in out_avals
    ]


def kernel(inter_times, h_ti, c_ti, cbar, W, b):
    sharded, in_names, out_names, out_avals, mesh = get_runner()
    cat = make_concat_inputs(inter_times, h_ti, c_ti, cbar, W, b)
    zeros = device_zeros(out_avals, mesh)
    out_arrs = sharded(*[cat[n] for n in in_names], *zeros)
    by_name = {n: np.asarray(a) for n, a in zip(out_names, out_arrs)}
    return tuple(
        by_name[n].astype(np.float32) for n in ["og", "hn", "cn", "cbn", "dec"]
    )


# revision 22
# speedup vs baseline: 1.0212x; 1.0212x over previous
"""Trainium2 Bass kernel for NeuralCTLSTM cell (B=65536, H=256, 7 gates).

Data-parallel over 8 NeuronCores (8192 batch rows each). Per core, per
128-row block:
  gates = h @ Wp^T + bp  (7 gates in one PSUM tile [128,1792], bf16 matmuls,
                          K=256 in 2 chunks + a K=1 ones-row for the bias)
  gate columns: [f, o, fbar, i, ibar, dneg, z] where the dneg slot holds the
  NEGATED decay-gate weights, so sigmoid drains 6 contiguous gates at once:
    sigma(-g_d) -> lns = ln(sigma(-g_d)) = -softplus(g_d) = -decay
    decay = -lns ; e = exp(-dt*decay) = exp(dt*lns)
  h^T needed by the PE comes straight from DRAM via DMA-transpose (bf16).
  c_after = cbar + (c-cbar)*e ; outputs og, hn, cn, cbn, dec (all bf16,
  host converts to fp32).

ACT table phasing: supergroups of 16 blocks; sigmoid/tanh (sigmoid set) for
the supergroup, then ln/exp (natural_log_exp set). tanh(c_after) of
supergroup g runs at the start of the sigmoid phase of g+1. Zero-valued
[P,1] tokens (passed as activation bias) enforce the phase edges.
"""

import os
import sys

sys.path.insert(0, "/opt/trn_rl_repo")

from contextlib import ExitStack

import numpy as np

NCORES = 8
B, H, G = 65536, 256, 7
P = 128
BL = B // NCORES           # rows per core
NBLK = BL // P             # 64 row-blocks per core
ST = 4                     # blocks per group (DMA/DVE batch)
NGRP = NBLK // ST          # 16 groups per core
GPS = 4                    # groups per supergroup
NSG = NGRP // GPS          # 4 supergroups
GH = G * H                 # 1792
SIGW = 6 * H               # 1536: f,o,fbar,i,ibar,dneg drained by one sigmoid

# gate column slots -> reference gate index (reference order:
# 0=input 1=forget 2=output 3=ibar 4=fbar 5=z 6=decay). Slot 5 (dneg) is
# negated on the host.
GATE_PERM = [1, 2, 4, 0, 3, 6, 5]

_NC = None


def build_nc():
    from concourse import bacc, mybir
    from concourse.tile import TileContext

    F32 = mybir.dt.float32
    BF16 = mybir.dt.bfloat16
    AF = mybir.ActivationFunctionType

    nc = bacc.Bacc("TRN2", target_bir_lowering=False, debug=False)

    # hT: host-pre-transposed h, [k-chunk, feature, batch-row] per core.
    # c/cbar: partition-major [row%128, block*H + col] so per-partition DMA
    # segments are contiguous (2 KB per block instead of 512 B lines).
    hT_d = nc.dram_tensor("hT", [2, P, BL], BF16, kind="ExternalInput")
    c_d = nc.dram_tensor("c", [P, NBLK * H], BF16, kind="ExternalInput")
    cb_d = nc.dram_tensor("cbar", [P, NBLK * H], BF16, kind="ExternalInput")
    dt_d = nc.dram_tensor("dts", [P, NBLK], F32, kind="ExternalInput")
    wt_d = nc.dram_tensor("wt", [2, P, GH], BF16, kind="ExternalInput")
    bia_d = nc.dram_tensor("bias", [1, GH], BF16, kind="ExternalInput")

    og_d = nc.dram_tensor("og", [BL, H], BF16, kind="ExternalOutput")
    hn_d = nc.dram_tensor("hn", [BL, H], BF16, kind="ExternalOutput")
    cn_d = nc.dram_tensor("cn", [BL, H], BF16, kind="ExternalOutput")
    cbn_d = nc.dram_tensor("cbn", [BL, H], BF16, kind="ExternalOutput")
    dec_d = nc.dram_tensor("dec", [BL, H], BF16, kind="ExternalOutput")

    og_r = og_d.rearrange("(n p) d -> n p d", p=P)
    hn_r = hn_d.rearrange("(n p) d -> n p d", p=P)
    cn_r = cn_d.rearrange("(n p) d -> n p d", p=P)
    cbn_r = cbn_d.rearrange("(n p) d -> n p d", p=P)
    dec_r = dec_d.rearrange("(n p) d -> n p d", p=P)

    def dsl(r, b0, n):  # dram slice of n row-blocks as [P, n, H]
        return r[b0 : b0 + n].rearrange("n p d -> p n d")

    MU = mybir.AluOpType.mult

    with TileContext(nc) as tc, ExitStack() as ctx:
        pool = lambda name, bufs, **kw: ctx.enter_context(
            tc.tile_pool(name=name, bufs=bufs, **kw)
        )
        const = pool("const", 1)
        hTp = pool("hTp", 2)
        cp = pool("cp", 2)
        cbp = pool("cbp", 2)
        Sp = pool("Sp", 6)
        Zp = pool("Zp", 6)
        lnsp = pool("lnsp", 4)
        einp = pool("einp", 2)
        ep = pool("ep", 4)
        cap = pool("cap", 6)
        tcap = pool("tcap", 2)
        tmpp = pool("tmpp", 2)
        hnp = pool("hnp", 2)
        cnp = pool("cnp", 2)
        cbnp = pool("cbnp", 2)
        ndecp = pool("ndecp", 2)
        tokp = pool("tokp", 4)
        pgp = pool("pgp", 2, space="PSUM")

        wt_sb = const.tile([P, 2, GH], BF16)
        nc.sync.dma_start(wt_sb[:], wt_d.rearrange("c k g -> k c g"))
        bia_sb = const.tile([1, GH], BF16)
        nc.sync.dma_start(bia_sb[:], bia_d[:, :])
        dts_sb = const.tile([P, NBLK], F32)
        nc.sync.dma_start(dts_sb[:], dt_d[:, :])
        ones_sb = const.tile([1, P], BF16)
        nc.vector.memset(ones_sb[:], 1.0)

        BANKS = [(0, 512), (512, 1024), (1024, 1536), (1536, 1792)]

        tokA = None        # gates SIG-phase ops of SG g after NLE ops of g-1
        prev = []          # [(S, ca, grp), ...] of previous supergroup

        SGB = GPS * ST  # blocks per supergroup (16)

        def issue_sg_inputs(g):
            hT = hTp.tile([P, 2, SGB * P], BF16)
            nc.sync.dma_start(
                hT[:],
                hT_d[:, :, g * SGB * P : (g + 1) * SGB * P].rearrange(
                    "c k r -> k c r"
                ),
            )
            c_sg = cp.tile([P, SGB, H], BF16)
            cb_sg = cbp.tile([P, SGB, H], BF16)
            nc.sync.dma_start(
                c_sg[:],
                c_d[:, g * SGB * H : (g + 1) * SGB * H].rearrange(
                    "k (n d) -> k n d", d=H
                ),
            )
            nc.sync.dma_start(
                cb_sg[:],
                cb_d[:, g * SGB * H : (g + 1) * SGB * H].rearrange(
                    "k (n d) -> k n d", d=H
                ),
            )
            return hT, c_sg, cb_sg

        cur_in = issue_sg_inputs(0)
        for g in range(NSG):
            # ---------------- SIG phase: matmuls + sigmoid/tanh drains ------
            hT, c_sg, cb_sg = cur_in
            nxt_in = issue_sg_inputs(g + 1) if g + 1 < NSG else None
            sg_S = []
            sg_Z = []
            tca_last = None
            for t in range(GPS):
                S = Sp.tile([P, ST, SIGW], BF16)
                Z = Zp.tile([P, ST, H], BF16)
                for j in range(ST):
                    jj = t * ST + j
                    Gp = pgp.tile([P, 2048], mybir.dt.float32)
                    for lo, hi in BANKS:
                        nc.tensor.matmul(
                            Gp[:, lo:hi], ones_sb[:], bia_sb[:, lo:hi],
                            start=True, stop=False,
                        )
                    for ck in range(2):
                        lhsT = hT[:, ck, jj * P : (jj + 1) * P]
                        for lo, hi in BANKS:
                            nc.tensor.matmul(
                                Gp[:, lo:hi], lhsT, wt_sb[:, ck, lo:hi],
                                start=False, stop=(ck == 1),
                            )
                    if tokA is not None:
                        nc.scalar.activation(
                            S[:, j, :], Gp[:, 0:SIGW], AF.Sigmoid, bias=tokA[:]
                        )
                        nc.scalar.activation(
                            Z[:, j, :], Gp[:, SIGW:GH], AF.Tanh, bias=tokA[:]
                        )
                    else:
                        nc.scalar.activation(S[:, j, :], Gp[:, 0:SIGW], AF.Sigmoid)
                        nc.scalar.activation(Z[:, j, :], Gp[:, SIGW:GH], AF.Tanh)
                sg_S.append(S)
                sg_Z.append(Z)
                nc.sync.dma_start(
                    dsl(og_r, (g * GPS + t) * ST, ST), S[:, :, H : 2 * H]
                )
                # previous supergroup, same group index: tanh(c_after) + hn
                if prev:
                    pS, pca, pgrp = prev[t]
                    tca = tcap.tile([P, ST, H], BF16)
                    if tokA is not None:
                        nc.scalar.activation(tca[:], pca[:], AF.Tanh, bias=tokA[:])
                    else:
                        nc.scalar.activation(tca[:], pca[:], AF.Tanh)
                    hn = hnp.tile([P, ST, H], BF16)
                    nc.vector.tensor_mul(hn[:], pS[:, :, H : 2 * H], tca[:])
                    nc.sync.dma_start(dsl(hn_r, pgrp * ST, ST), hn[:])
                    tca_last = tca

            # tokB: all NLE-set ACT ops of this SG wait on the SIG-set ops
            tokB = tokp.tile([P, 1], mybir.dt.float32)
            if tca_last is not None:
                nc.vector.scalar_tensor_tensor(
                    tokB[:], sg_Z[-1][:, ST - 1, 0:1], 0.0,
                    tca_last[:, ST - 1, 0:1], MU, MU,
                )
            else:
                nc.vector.tensor_scalar_mul(tokB[:], sg_Z[-1][:, ST - 1, 0:1], 0.0)

            # ---------------- NLE phase: ln/exp + elementwise + outputs -----
            # All Ln ops first, then all Exp ops: one natural_log table load
            # and one exp_and_others load per supergroup instead of 2 each.
            sg_lns = []
            for t in range(GPS):
                lns = lnsp.tile([P, ST, H], BF16)
                nc.scalar.activation(
                    lns[:], sg_S[t][:, :, 5 * H : 6 * H], AF.Ln, bias=tokB[:]
                )
                sg_lns.append(lns)
            sg_e = []
            for t in range(GPS):
                b0 = (g * GPS + t) * ST
                lns = sg_lns[t]
                ndec = ndecp.tile([P, ST, H], BF16)
                nc.vector.tensor_scalar_mul(ndec[:], lns[:], -1.0)
                nc.sync.dma_start(dsl(dec_r, b0, ST), ndec[:])
                ein = einp.tile([P, ST, H], BF16)
                for j in range(ST):
                    blk = b0 + j
                    nc.vector.tensor_scalar_mul(
                        ein[:, j, :], lns[:, j, :], dts_sb[:, blk : blk + 1]
                    )
                e = ep.tile([P, ST, H], BF16)
                nc.scalar.activation(e[:], ein[:], AF.Exp, bias=tokB[:])
                sg_e.append(e)
            prev_new = []
            for t in range(GPS):
                grp = g * GPS + t
                b0 = grp * ST
                S, Z, e = sg_S[t], sg_Z[t], sg_e[t]
                c_t = c_sg[:, t * ST : (t + 1) * ST, :]
                cb_t = cb_sg[:, t * ST : (t + 1) * ST, :]
                # elementwise chain (DVE, bf16 2x); two reused scratch tiles
                tA = tmpp.tile([P, ST, H], BF16)
                tB = tmpp.tile([P, ST, H], BF16)
                nc.vector.tensor_sub(tA[:], c_t, cb_t)          # d = c - cb
                nc.vector.tensor_mul(tB[:], tA[:], e[:])        # m = d * e
                ca = cap.tile([P, ST, H], BF16)
                nc.vector.tensor_add(ca[:], tB[:], cb_t)        # ca = m + cb
                nc.vector.tensor_mul(tA[:], S[:, :, 3 * H : 4 * H], Z[:])  # iz
                nc.vector.tensor_mul(tB[:], S[:, :, 0:H], ca[:])           # f*ca
                cn = cnp.tile([P, ST, H], BF16)
                nc.vector.tensor_add(cn[:], tB[:], tA[:])
                nc.sync.dma_start(dsl(cn_r, b0, ST), cn[:])
                nc.vector.tensor_mul(tA[:], S[:, :, 2 * H : 3 * H], cb_t)  # fbar*cb
                nc.vector.tensor_mul(tB[:], S[:, :, 4 * H : 5 * H], Z[:])  # ibar*z
                cbn = cbnp.tile([P, ST, H], BF16)
                nc.vector.tensor_add(cbn[:], tA[:], tB[:])
                nc.sync.dma_start(dsl(cbn_r, b0, ST), cbn[:])
                prev_new.append((S, ca, grp))

            prev = prev_new
            cur_in = nxt_in
            tokA = tokp.tile([P, 1], mybir.dt.float32)
            nc.vector.tensor_scalar_mul(tokA[:], sg_e[-1][:, ST - 1, 0:1], 0.0)

        # epilogue: hn for the final supergroup
        for t in range(GPS):
            pS, pca, pgrp = prev[t]
            tca = tcap.tile([P, ST, H], BF16)
            nc.scalar.activation(tca[:], pca[:], AF.Tanh, bias=tokA[:])
            hn = hnp.tile([P, ST, H], BF16)
            nc.vector.tensor_mul(hn[:], pS[:, :, H : 2 * H], tca[:])
            nc.sync.dma_start(dsl(hn_r, pgrp * ST, ST), hn[:])

    nc.compile()
    return nc


def prep_weights(W, b):
    """W [7,256,256] fp32, b [7,256] fp32 -> wt [2,128,1792] bf16, bias [1,1792] bf16."""
    import ml_dtypes

    Wp = np.ascontiguousarray(W[GATE_PERM]).astype(np.float32).copy()
    bp = np.ascontiguousarray(b[GATE_PERM]).astype(np.float32).copy()
    Wp[5] = -Wp[5]  # dneg slot: sigma(-g_d)
    bp[5] = -bp[5]
    # wt[ck, k, g*H+o] = Wp[g, o, ck*128+k]
    wt = np.transpose(Wp, (2, 0, 1)).reshape(H, GH)
    wt = np.ascontiguousarray(wt.reshape(2, P, GH)).astype(ml_dtypes.bfloat16)
    bia = bp.reshape(1, GH).astype(ml_dtypes.bfloat16)
    return wt, bia


_RUNNER = None


def _make_runner(nc):
    """Cached shard_map-jitted executor for nc across 8 cores."""
    import jax
    from jax.experimental.shard_map import shard_map
    from jax.sharding import Mesh, PartitionSpec

    from concourse import bass2jax, mybir

    bass2jax.install_neuronx_cc_hook()
    p = bass2jax._bass_exec_p

    part_name = nc.partition_id_tensor.name if nc.partition_id_tensor else None
    in_names, out_names, out_avals = [], [], []
    for alloc in nc.m.functions[0].allocations:
        if not isinstance(alloc, mybir.MemoryLocationSet):
            continue
        name = alloc.memorylocations[0].name
        if alloc.kind == "ExternalInput":
            if name != part_name:
                in_names.append(name)
        elif alloc.kind == "ExternalOutput":
            out_names.append(name)
            out_avals.append(
                jax.core.ShapedArray(tuple(alloc.tensor_shape), mybir.dt.np(alloc.dtype))
            )
    n_params = len(in_names)
    all_in = in_names + out_names
    if part_name is not None:
        all_in = all_in + [part_name]

    def _body(*args):
        operands = list(args)
        if part_name is not None:
            operands.append(bass2jax.partition_id_tensor())
        return tuple(
            p.bind(
                *operands,
                out_avals=tuple(out_avals),
                in_names=tuple(all_in),
                out_names=tuple(out_names),
                lowering_input_output_aliases=(),
                sim_require_finite=True,
                sim_require_nnan=True,
                nc=nc,
            )
        )

    devices = jax.devices()[:NCORES]
    mesh = Mesh(np.asarray(devices), ("core",))
    nin = n_params + len(out_names)
    sharded = jax.jit(
        shard_map(
            _body,
            mesh=mesh,
            in_specs=(PartitionSpec("core"),) * nin,
            out_specs=(PartitionSpec("core"),) * len(out_names),
            check_rep=False,
        ),
        donate_argnums=tuple(range(n_params, nin)),
        keep_unused=True,
    )
    return sharded, in_names, out_names, out_avals, mesh


def get_runner():
    global _NC, _RUNNER
    if _RUNNER is None:
        if _NC is None:
            _NC = build_nc()
        _RUNNER = _make_runner(_NC)
    return _RUNNER


def make_concat_inputs(inter_times, h_ti, c_ti, cbar, W, b):
    """Global (8*shape[0], ...) arrays keyed by dram tensor name."""
    import ml_dtypes

    BF = ml_dtypes.bfloat16
    inter_times = np.asarray(inter_times, dtype=np.float32)
    wt, bia = prep_weights(np.asarray(W, np.float32), np.asarray(b, np.float32))
    dts = np.ascontiguousarray(
        inter_times.reshape(NCORES, NBLK, P).transpose(0, 2, 1)
    ).reshape(NCORES * P, NBLK)
    # hT: [core, k-chunk, feature, row] -> concat [(core chunk), P, BL]
    h_bf = np.asarray(h_ti).astype(BF)
    hT = np.ascontiguousarray(
        h_bf.reshape(NCORES, BL, 2, P).transpose(0, 2, 3, 1)
    ).reshape(NCORES * 2, P, BL)
    # c/cbar: partition-major [core, row%128, block, col] -> [(core P), NBLK*H]
    def pmaj(x):
        xb = np.asarray(x).astype(BF)
        return np.ascontiguousarray(
            xb.reshape(NCORES, NBLK, P, H).transpose(0, 2, 1, 3)
        ).reshape(NCORES * P, NBLK * H)

    return {
        "hT": hT,
        "c": pmaj(c_ti),
        "cbar": pmaj(cbar),
        "dts": dts,
        "wt": np.ascontiguousarray(np.broadcast_to(wt, (NCORES,) + wt.shape)).reshape(
            NCORES * wt.shape[0], *wt.shape[1:]
        ),
        "bias": np.ascontiguousarray(
            np.broadcast_to(bia, (NCORES,) + bia.shape)
        ).reshape(NCORES * bia.shape[0], *bia.shape[1:]),
    }


def device_zeros(out_avals, mesh):
    import jax.numpy as jnp
    from jax.sharding import NamedSharding, PartitionSpec

    sh = NamedSharding(mesh, PartitionSpec("core"))
    return [
        jnp.zeros((NCORES * a.shape[0], *a.shape[1:]), a.dtype, device=sh)
        for a in out_avals
    ]


def kernel(inter_times, h_ti, c_ti, cbar, W, b):
    sharded, in_names, out_names, out_avals, mesh = get_runner()
    cat = make_concat_inputs(inter_times, h_ti, c_ti, cbar, W, b)
    zeros = device_zeros(out_avals, mesh)
    out_arrs = sharded(*[cat[n] for n in in_names], *zeros)
    by_name = {n: np.asarray(a) for n, a in zip(out_names, out_arrs)}
    return tuple(
        by_name[n].astype(np.float32) for n in ["og", "hn", "cn", "cbn", "dec"]
    )


# revision 24
# speedup vs baseline: 1.1446x; 1.1209x over previous
"""Trainium2 Bass kernel for NeuralCTLSTM cell (B=65536, H=256, 7 gates).

Data-parallel over 8 NeuronCores (8192 batch rows each). Per core, per
128-row block:
  gates = h @ Wp^T + bp  (7 gates in one PSUM tile [128,1792], bf16 matmuls,
                          K=256 in 2 chunks + a K=1 ones-row for the bias)
  gate columns: [f, o, fbar, i, ibar, dneg, z] where the dneg slot holds the
  NEGATED decay-gate weights, so sigmoid drains 6 contiguous gates at once:
    sigma(-g_d) -> lns = ln(sigma(-g_d)) = -softplus(g_d) = -decay
    decay = -lns ; e = exp(-dt*decay) = exp(dt*lns)
  h^T needed by the PE comes straight from DRAM via DMA-transpose (bf16).
  c_after = cbar + (c-cbar)*e ; outputs og, hn, cn, cbn, dec (all bf16,
  host converts to fp32).

ACT table phasing: supergroups of 16 blocks; sigmoid/tanh (sigmoid set) for
the supergroup, then ln/exp (natural_log_exp set). tanh(c_after) of
supergroup g runs at the start of the sigmoid phase of g+1. Zero-valued
[P,1] tokens (passed as activation bias) enforce the phase edges.
"""

import os
import sys

sys.path.insert(0, "/opt/trn_rl_repo")

from contextlib import ExitStack

import numpy as np

NCORES = 8
B, H, G = 65536, 256, 7
P = 128
BL = B // NCORES           # rows per core
NBLK = BL // P             # 64 row-blocks per core
ST = 4                     # blocks per group (DMA/DVE batch)
NGRP = NBLK // ST          # 16 groups per core
GPS = 4                    # groups per supergroup
NSG = NGRP // GPS          # 4 supergroups
GH = G * H                 # 1792
SIGW = 6 * H               # 1536: f,o,fbar,i,ibar,dneg drained by one sigmoid

# gate column slots -> reference gate index (reference order:
# 0=input 1=forget 2=output 3=ibar 4=fbar 5=z 6=decay). Slot 5 (dneg) is
# negated on the host.
GATE_PERM = [1, 2, 4, 0, 3, 6, 5]

_NC = None


def build_nc():
    from concourse import bacc, mybir
    from concourse.tile import TileContext

    F32 = mybir.dt.float32
    BF16 = mybir.dt.bfloat16
    AF = mybir.ActivationFunctionType

    nc = bacc.Bacc("TRN2", target_bir_lowering=False, debug=False)

    # hT: host-pre-transposed h, [k-chunk, feature, batch-row] per core.
    # c/cbar: partition-major [row%128, block*H + col] so per-partition DMA
    # segments are contiguous (2 KB per block instead of 512 B lines).
    hT_d = nc.dram_tensor("hT", [2, P, BL], BF16, kind="ExternalInput")
    c_d = nc.dram_tensor("c", [P, NBLK * H], BF16, kind="ExternalInput")
    cb_d = nc.dram_tensor("cbar", [P, NBLK * H], BF16, kind="ExternalInput")
    dt_d = nc.dram_tensor("dts", [P, NBLK], F32, kind="ExternalInput")
    wt_d = nc.dram_tensor("wt", [2, P, GH], BF16, kind="ExternalInput")
    bia_d = nc.dram_tensor("bias", [1, GH], BF16, kind="ExternalInput")

    og_d = nc.dram_tensor("og", [BL, H], BF16, kind="ExternalOutput")
    hn_d = nc.dram_tensor("hn", [BL, H], BF16, kind="ExternalOutput")
    cn_d = nc.dram_tensor("cn", [BL, H], BF16, kind="ExternalOutput")
    cbn_d = nc.dram_tensor("cbn", [BL, H], BF16, kind="ExternalOutput")
    dec_d = nc.dram_tensor("dec", [BL, H], BF16, kind="ExternalOutput")

    og_r = og_d.rearrange("(n p) d -> n p d", p=P)
    hn_r = hn_d.rearrange("(n p) d -> n p d", p=P)
    cn_r = cn_d.rearrange("(n p) d -> n p d", p=P)
    cbn_r = cbn_d.rearrange("(n p) d -> n p d", p=P)
    dec_r = dec_d.rearrange("(n p) d -> n p d", p=P)

    def dsl(r, b0, n):  # dram slice of n row-blocks as [P, n, H]
        return r[b0 : b0 + n].rearrange("n p d -> p n d")

    MU = mybir.AluOpType.mult

    with TileContext(nc) as tc, ExitStack() as ctx:
        pool = lambda name, bufs, **kw: ctx.enter_context(
            tc.tile_pool(name=name, bufs=bufs, **kw)
        )
        const = pool("const", 1)
        hTp = pool("hTp", 2)
        cp = pool("cp", 2)
        cbp = pool("cbp", 2)
        Sp = pool("Sp", 6)
        Zp = pool("Zp", 6)
        lnsp = pool("lnsp", 4)
        einp = pool("einp", 2)
        ep = pool("ep", 4)
        cap = pool("cap", 6)
        tcap = pool("tcap", 2)
        tmpp = pool("tmpp", 2)
        hnp = pool("hnp", 2)
        cnp = pool("cnp", 2)
        cbnp = pool("cbnp", 2)
        ndecp = pool("ndecp", 2)
        tokp = pool("tokp", 4)
        pgp = pool("pgp", 2, space="PSUM")

        wt_sb = const.tile([P, 2, GH], BF16)
        nc.sync.dma_start(wt_sb[:], wt_d.rearrange("c k g -> k c g"))
        bia_sb = const.tile([1, GH], BF16)
        nc.sync.dma_start(bia_sb[:], bia_d[:, :])
        dts_sb = const.tile([P, NBLK], F32)
        nc.sync.dma_start(dts_sb[:], dt_d[:, :])
        ones_sb = const.tile([1, P], BF16)
        nc.vector.memset(ones_sb[:], 1.0)

        BANKS = [(0, 512), (512, 1024), (1024, 1536), (1536, 1792)]

        tokA = None        # gates SIG-phase ops of SG g after NLE ops of g-1
        prev = []          # [(S, ca, grp), ...] of previous supergroup

        SGB = GPS * ST  # blocks per supergroup (16)

        def issue_sg_inputs(g):
            hT = hTp.tile([P, 2, SGB * P], BF16)
            nc.sync.dma_start(
                hT[:],
                hT_d[:, :, g * SGB * P : (g + 1) * SGB * P].rearrange(
                    "c k r -> k c r"
                ),
            )
            c_sg = cp.tile([P, SGB, H], BF16)
            cb_sg = cbp.tile([P, SGB, H], BF16)
            nc.sync.dma_start(
                c_sg[:],
                c_d[:, g * SGB * H : (g + 1) * SGB * H].rearrange(
                    "k (n d) -> k n d", d=H
                ),
            )
            nc.sync.dma_start(
                cb_sg[:],
                cb_d[:, g * SGB * H : (g + 1) * SGB * H].rearrange(
                    "k (n d) -> k n d", d=H
                ),
            )
            return hT, c_sg, cb_sg

        cur_in = issue_sg_inputs(0)
        for g in range(NSG):
            # ---------------- SIG phase: matmuls + sigmoid/tanh drains ------
            hT, c_sg, cb_sg = cur_in
            nxt_in = issue_sg_inputs(g + 1) if g + 1 < NSG else None
            sg_S = []
            sg_Z = []
            tca_last = None
            for t in range(GPS):
                S = Sp.tile([P, ST, SIGW], BF16)
                Z = Zp.tile([P, ST, H], BF16)
                for j in range(ST):
                    jj = t * ST + j
                    Gp = pgp.tile([P, 2048], mybir.dt.float32)
                    for lo, hi in BANKS:
                        nc.tensor.matmul(
                            Gp[:, lo:hi], ones_sb[:], bia_sb[:, lo:hi],
                            start=True, stop=False,
                        )
                    for ck in range(2):
                        lhsT = hT[:, ck, jj * P : (jj + 1) * P]
                        for lo, hi in BANKS:
                            nc.tensor.matmul(
                                Gp[:, lo:hi], lhsT, wt_sb[:, ck, lo:hi],
                                start=False, stop=(ck == 1),
                            )
                    if tokA is not None:
                        nc.scalar.activation(
                            S[:, j, :], Gp[:, 0:SIGW], AF.Sigmoid, bias=tokA[:]
                        )
                        nc.scalar.activation(
                            Z[:, j, :], Gp[:, SIGW:GH], AF.Tanh, bias=tokA[:]
                        )
                    else:
                        nc.scalar.activation(S[:, j, :], Gp[:, 0:SIGW], AF.Sigmoid)
                        nc.scalar.activation(Z[:, j, :], Gp[:, SIGW:GH], AF.Tanh)
                sg_S.append(S)
                sg_Z.append(Z)
                nc.sync.dma_start(
                    dsl(og_r, (g * GPS + t) * ST, ST), S[:, :, H : 2 * H]
                )
                # previous supergroup, same group index: tanh(c_after) + hn
                if prev:
                    pS, pca, pgrp = prev[t]
                    tca = tcap.tile([P, ST, H], BF16)
                    if tokA is not None:
                        nc.scalar.activation(tca[:], pca[:], AF.Tanh, bias=tokA[:])
                    else:
                        nc.scalar.activation(tca[:], pca[:], AF.Tanh)
                    hn = hnp.tile([P, ST, H], BF16)
                    nc.vector.tensor_mul(hn[:], pS[:, :, H : 2 * H], tca[:])
                    nc.sync.dma_start(dsl(hn_r, pgrp * ST, ST), hn[:])
                    tca_last = tca

            # tokB: all NLE-set ACT ops of this SG wait on the SIG-set ops
            tokB = tokp.tile([P, 1], mybir.dt.float32)
            if tca_last is not None:
                nc.vector.scalar_tensor_tensor(
                    tokB[:], sg_Z[-1][:, ST - 1, 0:1], 0.0,
                    tca_last[:, ST - 1, 0:1], MU, MU,
                )
            else:
                nc.vector.tensor_scalar_mul(tokB[:], sg_Z[-1][:, ST - 1, 0:1], 0.0)

            # ---------------- NLE phase: ln/exp + elementwise + outputs -----
            # All Ln ops first, then all Exp ops: one natural_log table load
            # and one exp_and_others load per supergroup instead of 2 each.
            sg_lns = []
            for t in range(GPS):
                lns = lnsp.tile([P, ST, H], BF16)
                nc.scalar.activation(
                    lns[:], sg_S[t][:, :, 5 * H : 6 * H], AF.Ln, bias=tokB[:]
                )
                sg_lns.append(lns)
            sg_e = []
            for t in range(GPS):
                b0 = (g * GPS + t) * ST
                lns = sg_lns[t]
                ndec = ndecp.tile([P, ST, H], BF16)
                nc.vector.tensor_scalar_mul(ndec[:], lns[:], -1.0)
                nc.sync.dma_start(dsl(dec_r, b0, ST), ndec[:])
                ein = einp.tile([P, ST, H], BF16)
                for j in range(ST):
                    blk = b0 + j
                    nc.vector.tensor_scalar_mul(
                        ein[:, j, :], lns[:, j, :], dts_sb[:, blk : blk + 1]
                    )
                e = ep.tile([P, ST, H], BF16)
                nc.scalar.activation(e[:], ein[:], AF.Exp, bias=tokB[:])
                sg_e.append(e)
            # tokA must fire as soon as the last Exp lands — BEFORE the DVE
            # chains below — or the next supergroup's sigmoids (gated on it)
            # sit behind ~27us of chain work in the in-order DVE queue.
            tokA = tokp.tile([P, 1], mybir.dt.float32)
            nc.vector.tensor_scalar_mul(tokA[:], sg_e[-1][:, ST - 1, 0:1], 0.0)
            prev_new = []
            for t in range(GPS):
                grp = g * GPS + t
                b0 = grp * ST
                S, Z, e = sg_S[t], sg_Z[t], sg_e[t]
                c_t = c_sg[:, t * ST : (t + 1) * ST, :]
                cb_t = cb_sg[:, t * ST : (t + 1) * ST, :]
                # elementwise chain (DVE, bf16 2x); two reused scratch tiles
                tA = tmpp.tile([P, ST, H], BF16)
                tB = tmpp.tile([P, ST, H], BF16)
                nc.vector.tensor_sub(tA[:], c_t, cb_t)          # d = c - cb
                nc.vector.tensor_mul(tB[:], tA[:], e[:])        # m = d * e
                ca = cap.tile([P, ST, H], BF16)
                nc.vector.tensor_add(ca[:], tB[:], cb_t)        # ca = m + cb
                nc.vector.tensor_mul(tA[:], S[:, :, 3 * H : 4 * H], Z[:])  # iz
                nc.vector.tensor_mul(tB[:], S[:, :, 0:H], ca[:])           # f*ca
                cn = cnp.tile([P, ST, H], BF16)
                nc.vector.tensor_add(cn[:], tB[:], tA[:])
                nc.sync.dma_start(dsl(cn_r, b0, ST), cn[:])
                nc.vector.tensor_mul(tA[:], S[:, :, 2 * H : 3 * H], cb_t)  # fbar*cb
                nc.vector.tensor_mul(tB[:], S[:, :, 4 * H : 5 * H], Z[:])  # ibar*z
                cbn = cbnp.tile([P, ST, H], BF16)
                nc.vector.tensor_add(cbn[:], tA[:], tB[:])
                nc.sync.dma_start(dsl(cbn_r, b0, ST), cbn[:])
                prev_new.append((S, ca, grp))

            prev = prev_new
            cur_in = nxt_in

        # epilogue: hn for the final supergroup
        for t in range(GPS):
            pS, pca, pgrp = prev[t]
            tca = tcap.tile([P, ST, H], BF16)
            nc.scalar.activation(tca[:], pca[:], AF.Tanh, bias=tokA[:])
            hn = hnp.tile([P, ST, H], BF16)
            nc.vector.tensor_mul(hn[:], pS[:, :, H : 2 * H], tca[:])
            nc.sync.dma_start(dsl(hn_r, pgrp * ST, ST), hn[:])

    nc.compile()
    return nc


def prep_weights(W, b):
    """W [7,256,256] fp32, b [7,256] fp32 -> wt [2,128,1792] bf16, bias [1,1792] bf16."""
    import ml_dtypes

    Wp = np.ascontiguousarray(W[GATE_PERM]).astype(np.float32).copy()
    bp = np.ascontiguousarray(b[GATE_PERM]).astype(np.float32).copy()
    Wp[5] = -Wp[5]  # dneg slot: sigma(-g_d)
    bp[5] = -bp[5]
    # wt[ck, k, g*H+o] = Wp[g, o, ck*128+k]
    wt = np.transpose(Wp, (2, 0, 1)).reshape(H, GH)
    wt = np.ascontiguousarray(wt.reshape(2, P, GH)).astype(ml_dtypes.bfloat16)
    bia = bp.reshape(1, GH).astype(ml_dtypes.bfloat16)
    return wt, bia


_RUNNER = None


def _make_runner(nc):
    """Cached shard_map-jitted executor for nc across 8 cores."""
    import jax
    from jax.experimental.shard_map import shard_map
    from jax.sharding import Mesh, PartitionSpec

    from concourse import bass2jax, mybir

    bass2jax.install_neuronx_cc_hook()
    p = bass2jax._bass_exec_p

    part_name = nc.partition_id_tensor.name if nc.partition_id_tensor else None
    in_names, out_names, out_avals = [], [], []
    for alloc in nc.m.functions[0].allocations:
        if not isinstance(alloc, mybir.MemoryLocationSet):
            continue
        name = alloc.memorylocations[0].name
        if alloc.kind == "ExternalInput":
            if name != part_name:
                in_names.append(name)
        elif alloc.kind == "ExternalOutput":
            out_names.append(name)
            out_avals.append(
                jax.core.ShapedArray(tuple(alloc.tensor_shape), mybir.dt.np(alloc.dtype))
            )
    n_params = len(in_names)
    all_in = in_names + out_names
    if part_name is not None:
        all_in = all_in + [part_name]

    def _body(*args):
        operands = list(args)
        if part_name is not None:
            operands.append(bass2jax.partition_id_tensor())
        return tuple(
            p.bind(
                *operands,
                out_avals=tuple(out_avals),
                in_names=tuple(all_in),
                out_names=tuple(out_names),
                lowering_input_output_aliases=(),
                sim_require_finite=True,
                sim_require_nnan=True,
                nc=nc,
            )
        )

    devices = jax.devices()[:NCORES]
    mesh = Mesh(np.asarray(devices), ("core",))
    nin = n_params + len(out_names)
    sharded = jax.jit(
        shard_map(
            _body,
            mesh=mesh,
            in_specs=(PartitionSpec("core"),) * nin,
            out_specs=(PartitionSpec("core"),) * len(out_names),
            check_rep=False,
        ),
        donate_argnums=tuple(range(n_params, nin)),
        keep_unused=True,
    )
    return sharded, in_names, out_names, out_avals, mesh


def get_runner():
    global _NC, _RUNNER
    if _RUNNER is None:
        if _NC is None:
            _NC = build_nc()
        _RUNNER = _make_runner(_NC)
    return _RUNNER


def make_concat_inputs(inter_times, h_ti, c_ti, cbar, W, b):
    """Global (8*shape[0], ...) arrays keyed by dram tensor name."""
    import ml_dtypes

    BF = ml_dtypes.bfloat16
    inter_times = np.asarray(inter_times, dtype=np.float32)
    wt, bia = prep_weights(np.asarray(W, np.float32), np.asarray(b, np.float32))
    dts = np.ascontiguousarray(
        inter_times.reshape(NCORES, NBLK, P).transpose(0, 2, 1)
    ).reshape(NCORES * P, NBLK)
    # hT: [core, k-chunk, feature, row] -> concat [(core chunk), P, BL]
    h_bf = np.asarray(h_ti).astype(BF)
    hT = np.ascontiguousarray(
        h_bf.reshape(NCORES, BL, 2, P).transpose(0, 2, 3, 1)
    ).reshape(NCORES * 2, P, BL)
    # c/cbar: partition-major [core, row%128, block, col] -> [(core P), NBLK*H]
    def pmaj(x):
        xb = np.asarray(x).astype(BF)
        return np.ascontiguousarray(
            xb.reshape(NCORES, NBLK, P, H).transpose(0, 2, 1, 3)
        ).reshape(NCORES * P, NBLK * H)

    return {
        "hT": hT,
        "c": pmaj(c_ti),
        "cbar": pmaj(cbar),
        "dts": dts,
        "wt": np.ascontiguousarray(np.broadcast_to(wt, (NCORES,) + wt.shape)).reshape(
            NCORES * wt.shape[0], *wt.shape[1:]
        ),
        "bias": np.ascontiguousarray(
            np.broadcast_to(bia, (NCORES,) + bia.shape)
        ).reshape(NCORES * bia.shape[0], *bia.shape[1:]),
    }


def device_zeros(out_avals, mesh):
    import jax.numpy as jnp
    from jax.sharding import NamedSharding, PartitionSpec

    sh = NamedSharding(mesh, PartitionSpec("core"))
    return [
        jnp.zeros((NCORES * a.shape[0], *a.shape[1:]), a.dtype, device=sh)
        for a in out_avals
    ]


def kernel(inter_times, h_ti, c_ti, cbar, W, b):
    sharded, in_names, out_names, out_avals, mesh = get_runner()
    cat = make_concat_inputs(inter_times, h_ti, c_ti, cbar, W, b)
    zeros = device_zeros(out_avals, mesh)
    out_arrs = sharded(*[cat[n] for n in in_names], *zeros)
    by_name = {n: np.asarray(a) for n, a in zip(out_names, out_arrs)}
    return tuple(
        by_name[n].astype(np.float32) for n in ["og", "hn", "cn", "cbn", "dec"]
    )


# revision 28
# speedup vs baseline: 1.2408x; 1.0841x over previous
"""Trainium2 Bass kernel for NeuralCTLSTM cell (B=65536, H=256, 7 gates).

Data-parallel over 8 NeuronCores (8192 batch rows each). Per core, per
128-row block:
  gates = h @ Wp^T + bp  (7 gates in one PSUM tile [128,1792], bf16 matmuls,
                          K=256 in 2 chunks + a K=1 ones-row for the bias)
  gate columns: [f, o, fbar, i, ibar, dneg, z] where the dneg slot holds the
  NEGATED decay-gate weights, so sigmoid drains 6 contiguous gates at once:
    sigma(-g_d) -> lns = ln(sigma(-g_d)) = -softplus(g_d) = -decay
    decay = -lns ; e = exp(-dt*decay) = exp(dt*lns)
  h^T needed by the PE comes straight from DRAM via DMA-transpose (bf16).
  c_after = cbar + (c-cbar)*e ; outputs og, hn, cn, cbn, dec (all bf16,
  host converts to fp32).

ACT table phasing: supergroups of 16 blocks; sigmoid/tanh (sigmoid set) for
the supergroup, then ln/exp (natural_log_exp set). tanh(c_after) of
supergroup g runs at the start of the sigmoid phase of g+1. Zero-valued
[P,1] tokens (passed as activation bias) enforce the phase edges.
"""

import os
import sys

sys.path.insert(0, "/opt/trn_rl_repo")

from contextlib import ExitStack

import numpy as np

NCORES = 8
B, H, G = 65536, 256, 7
P = 128
BL = B // NCORES           # rows per core
NBLK = BL // P             # 64 row-blocks per core
ST = 4                     # blocks per group (DMA/DVE batch)
NGRP = NBLK // ST          # 16 groups per core
GPS = 4                    # groups per supergroup
NSG = NGRP // GPS          # 4 supergroups
GH = G * H                 # 1792
SIGW = 6 * H               # 1536: f,o,fbar,i,ibar,dneg drained by one sigmoid

# gate column slots -> reference gate index (reference order:
# 0=input 1=forget 2=output 3=ibar 4=fbar 5=z 6=decay). Slot 5 (dneg) is
# negated on the host.
GATE_PERM = [1, 2, 4, 0, 3, 6, 5]

_NC = None


def build_nc():
    from concourse import bacc, mybir
    from concourse.tile import TileContext

    F32 = mybir.dt.float32
    F16 = mybir.dt.float16
    AF = mybir.ActivationFunctionType

    nc = bacc.Bacc("TRN2", target_bir_lowering=False, debug=False)

    # hT: host-pre-transposed h, [k-chunk, feature, batch-row] per core.
    # c/cbar: partition-major [row%128, block*H + col] so per-partition DMA
    # segments are contiguous (2 KB per block instead of 512 B lines).
    hT_d = nc.dram_tensor("hT", [2, P, BL], F16, kind="ExternalInput")
    c_d = nc.dram_tensor("c", [P, NBLK * H], F16, kind="ExternalInput")
    cb_d = nc.dram_tensor("cbar", [P, NBLK * H], F16, kind="ExternalInput")
    dt_d = nc.dram_tensor("dts", [P, NBLK], F32, kind="ExternalInput")
    wt_d = nc.dram_tensor("wt", [2, P, GH], F16, kind="ExternalInput")
    bia_d = nc.dram_tensor("bias", [1, GH], F16, kind="ExternalInput")

    og_d = nc.dram_tensor("og", [BL, H], F16, kind="ExternalOutput")
    hn_d = nc.dram_tensor("hn", [BL, H], F16, kind="ExternalOutput")
    cn_d = nc.dram_tensor("cn", [BL, H], F16, kind="ExternalOutput")
    cbn_d = nc.dram_tensor("cbn", [BL, H], F16, kind="ExternalOutput")
    dec_d = nc.dram_tensor("dec", [BL, H], F16, kind="ExternalOutput")

    og_r = og_d.rearrange("(n p) d -> n p d", p=P)
    hn_r = hn_d.rearrange("(n p) d -> n p d", p=P)
    cn_r = cn_d.rearrange("(n p) d -> n p d", p=P)
    cbn_r = cbn_d.rearrange("(n p) d -> n p d", p=P)
    dec_r = dec_d.rearrange("(n p) d -> n p d", p=P)

    def dsl(r, b0, n):  # dram slice of n row-blocks as [P, n, H]
        return r[b0 : b0 + n].rearrange("n p d -> p n d")

    MU = mybir.AluOpType.mult

    with TileContext(nc) as tc, ExitStack() as ctx:
        pool = lambda name, bufs, **kw: ctx.enter_context(
            tc.tile_pool(name=name, bufs=bufs, **kw)
        )
        const = pool("const", 1)
        hTp = pool("hTp", 2)
        cp = pool("cp", 2)
        cbp = pool("cbp", 2)
        Sp = pool("Sp", 6)
        ztp = pool("ztp", 2)
        lnsp = pool("lnsp", 4)
        einp = pool("einp", 2)
        ep = pool("ep", 4)
        cap = pool("cap", 6)
        tcap = pool("tcap", 2)
        tmpp = pool("tmpp", 2)
        hnp = pool("hnp", 2)
        cnp = pool("cnp", 2)
        cbnp = pool("cbnp", 2)
        ndecp = pool("ndecp", 2)
        tokp = pool("tokp", 4)
        pgp = pool("pgp", 2, space="PSUM")

        wt_sb = const.tile([P, 2, GH], F16)
        nc.sync.dma_start(wt_sb[:], wt_d.rearrange("c k g -> k c g"))
        bia_sb = const.tile([1, GH], F16)
        nc.sync.dma_start(bia_sb[:], bia_d[:, :])
        dts_sb = const.tile([P, NBLK], F32)
        nc.sync.dma_start(dts_sb[:], dt_d[:, :])
        ones_sb = const.tile([1, P], F16)
        nc.vector.memset(ones_sb[:], 1.0)

        BANKS = [(0, 512), (512, 1024), (1024, 1536), (1536, 1792)]

        tokA = None        # gates SIG-phase ops of SG g after NLE ops of g-1
        prev = []          # [(S, ca, grp), ...] of previous supergroup

        SGB = GPS * ST  # blocks per supergroup (16)

        def issue_sg_inputs(g):
            hT = hTp.tile([P, 2, SGB * P], F16)
            nc.sync.dma_start(
                hT[:],
                hT_d[:, :, g * SGB * P : (g + 1) * SGB * P].rearrange(
                    "c k r -> k c r"
                ),
            )
            c_sg = cp.tile([P, SGB, H], F16)
            cb_sg = cbp.tile([P, SGB, H], F16)
            nc.sync.dma_start(
                c_sg[:],
                c_d[:, g * SGB * H : (g + 1) * SGB * H].rearrange(
                    "k (n d) -> k n d", d=H
                ),
            )
            nc.sync.dma_start(
                cb_sg[:],
                cb_d[:, g * SGB * H : (g + 1) * SGB * H].rearrange(
                    "k (n d) -> k n d", d=H
                ),
            )
            return hT, c_sg, cb_sg

        cur_in = issue_sg_inputs(0)
        for g in range(NSG):
            # ---------------- SIG phase: matmuls + sigmoid/tanh drains ------
            hT, c_sg, cb_sg = cur_in
            nxt_in = issue_sg_inputs(g + 1) if g + 1 < NSG else None
            sg_S = []
            tca_last = None
            for t in range(GPS):
                S = Sp.tile([P, ST, GH], F16)
                for j in range(ST):
                    jj = t * ST + j
                    Gp = pgp.tile([P, 2048], mybir.dt.float32)
                    for lo, hi in BANKS:
                        nc.tensor.matmul(
                            Gp[:, lo:hi], ones_sb[:], bia_sb[:, lo:hi],
                            start=True, stop=False,
                        )
                    for ck in range(2):
                        lhsT = hT[:, ck, jj * P : (jj + 1) * P]
                        for lo, hi in BANKS:
                            nc.tensor.matmul(
                                Gp[:, lo:hi], lhsT, wt_sb[:, ck, lo:hi],
                                start=False, stop=(ck == 1),
                            )
                    if tokA is not None:
                        nc.scalar.activation(
                            S[:, j, :], Gp[:, 0:GH], AF.Sigmoid, bias=tokA[:]
                        )
                    else:
                        nc.scalar.activation(S[:, j, :], Gp[:, 0:GH], AF.Sigmoid)
                sg_S.append(S)
                nc.sync.dma_start(
                    dsl(og_r, (g * GPS + t) * ST, ST), S[:, :, H : 2 * H]
                )
                # previous supergroup, same group index: tanh(c_after) + hn
                if prev:
                    pS, pca, pgrp = prev[t]
                    tca = tcap.tile([P, ST, H], F16)
                    if tokA is not None:
                        nc.scalar.activation(tca[:], pca[:], AF.Tanh, bias=tokA[:])
                    else:
                        nc.scalar.activation(tca[:], pca[:], AF.Tanh)
                    hn = hnp.tile([P, ST, H], F16)
                    nc.vector.tensor_mul(hn[:], pS[:, :, H : 2 * H], tca[:])
                    nc.sync.dma_start(dsl(hn_r, pgrp * ST, ST), hn[:])
                    tca_last = tca

            # tokB: all NLE-set ACT ops of this SG wait on the SIG-set ops
            tokB = tokp.tile([P, 1], mybir.dt.float32)
            zsrc = sg_S[-1][:, ST - 1, 6 * H : 6 * H + 1]
            if tca_last is not None:
                nc.vector.scalar_tensor_tensor(
                    tokB[:], zsrc, 0.0, tca_last[:, ST - 1, 0:1], MU, MU,
                )
            else:
                nc.vector.tensor_scalar_mul(tokB[:], zsrc, 0.0)

            # ---------------- NLE phase: ln/exp + elementwise + outputs -----
            # All Ln ops first, then all Exp ops: one natural_log table load
            # and one exp_and_others load per supergroup instead of 2 each.
            sg_lns = []
            for t in range(GPS):
                lns = lnsp.tile([P, ST, H], F16)
                nc.scalar.activation(
                    lns[:], sg_S[t][:, :, 5 * H : 6 * H], AF.Ln, bias=tokB[:]
                )
                sg_lns.append(lns)
            sg_e = []
            for t in range(GPS):
                b0 = (g * GPS + t) * ST
                lns = sg_lns[t]
                ndec = ndecp.tile([P, ST, H], F16)
                nc.vector.tensor_scalar_mul(ndec[:], lns[:], -1.0)
                nc.sync.dma_start(dsl(dec_r, b0, ST), ndec[:])
                ein = einp.tile([P, ST, H], F16)
                for j in range(ST):
                    blk = b0 + j
                    nc.vector.tensor_scalar_mul(
                        ein[:, j, :], lns[:, j, :], dts_sb[:, blk : blk + 1]
                    )
                e = ep.tile([P, ST, H], F16)
                nc.scalar.activation(e[:], ein[:], AF.Exp, bias=tokB[:])
                sg_e.append(e)
            # tokA must fire as soon as the last Exp lands — BEFORE the DVE
            # chains below — or the next supergroup's sigmoids (gated on it)
            # sit behind ~27us of chain work in the in-order DVE queue.
            tokA = tokp.tile([P, 1], mybir.dt.float32)
            nc.vector.tensor_scalar_mul(tokA[:], sg_e[-1][:, ST - 1, 0:1], 0.0)
            prev_new = []
            for t in range(GPS):
                grp = g * GPS + t
                b0 = grp * ST
                S, e = sg_S[t], sg_e[t]
                zt = ztp.tile([P, ST, H], F16)
                nc.vector.tensor_scalar(
                    zt[:], S[:, :, 6 * H : 7 * H], 2.0, -1.0,
                    mybir.AluOpType.mult, mybir.AluOpType.add,
                )
                c_t = c_sg[:, t * ST : (t + 1) * ST, :]
                cb_t = cb_sg[:, t * ST : (t + 1) * ST, :]
                # elementwise chain (DVE, bf16 2x); two reused scratch tiles
                tA = tmpp.tile([P, ST, H], F16)
                tB = tmpp.tile([P, ST, H], F16)
                nc.vector.tensor_sub(tA[:], c_t, cb_t)          # d = c - cb
                nc.vector.tensor_mul(tB[:], tA[:], e[:])        # m = d * e
                ca = cap.tile([P, ST, H], F16)
                nc.vector.tensor_add(ca[:], tB[:], cb_t)        # ca = m + cb
                nc.vector.tensor_mul(tA[:], S[:, :, 3 * H : 4 * H], zt[:])  # iz
                nc.vector.tensor_mul(tB[:], S[:, :, 0:H], ca[:])           # f*ca
                cn = cnp.tile([P, ST, H], F16)
                nc.vector.tensor_add(cn[:], tB[:], tA[:])
                nc.sync.dma_start(dsl(cn_r, b0, ST), cn[:])
                nc.vector.tensor_mul(tA[:], S[:, :, 2 * H : 3 * H], cb_t)  # fbar*cb
                nc.vector.tensor_mul(tB[:], S[:, :, 4 * H : 5 * H], zt[:])  # ibar*z
                cbn = cbnp.tile([P, ST, H], F16)
                nc.vector.tensor_add(cbn[:], tA[:], tB[:])
                nc.sync.dma_start(dsl(cbn_r, b0, ST), cbn[:])
                prev_new.append((S, ca, grp))

            prev = prev_new
            cur_in = nxt_in

        # epilogue: hn for the final supergroup
        for t in range(GPS):
            pS, pca, pgrp = prev[t]
            tca = tcap.tile([P, ST, H], F16)
            nc.scalar.activation(tca[:], pca[:], AF.Tanh, bias=tokA[:])
            hn = hnp.tile([P, ST, H], F16)
            nc.vector.tensor_mul(hn[:], pS[:, :, H : 2 * H], tca[:])
            nc.sync.dma_start(dsl(hn_r, pgrp * ST, ST), hn[:])

    nc.compile()
    return nc


def prep_weights(W, b):
    """W [7,256,256] fp32, b [7,256] fp32 -> wt [2,128,1792] fp16, bias [1,1792] fp16.

    Slot 5 (decay gate) negated so sigma(-g_d) comes out of the big sigmoid;
    slot 6 (z gate) doubled so tanh(g_z) = 2*sigma(2 g_z) - 1.
    """
    Wp = np.ascontiguousarray(W[GATE_PERM]).astype(np.float32).copy()
    bp = np.ascontiguousarray(b[GATE_PERM]).astype(np.float32).copy()
    Wp[5] = -Wp[5]  # dneg slot: sigma(-g_d)
    bp[5] = -bp[5]
    Wp[6] = 2.0 * Wp[6]  # z slot: sigma(2 g_z)
    bp[6] = 2.0 * bp[6]
    # wt[ck, k, g*H+o] = Wp[g, o, ck*128+k]
    wt = np.transpose(Wp, (2, 0, 1)).reshape(H, GH)
    wt = np.ascontiguousarray(wt.reshape(2, P, GH)).astype(np.float16)
    bia = bp.reshape(1, GH).astype(np.float16)
    return wt, bia


_RUNNER = None


def _make_runner(nc):
    """Cached shard_map-jitted executor for nc across 8 cores."""
    import jax
    from jax.experimental.shard_map import shard_map
    from jax.sharding import Mesh, PartitionSpec

    from concourse import bass2jax, mybir

    bass2jax.install_neuronx_cc_hook()
    p = bass2jax._bass_exec_p

    part_name = nc.partition_id_tensor.name if nc.partition_id_tensor else None
    in_names, out_names, out_avals = [], [], []
    for alloc in nc.m.functions[0].allocations:
        if not isinstance(alloc, mybir.MemoryLocationSet):
            continue
        name = alloc.memorylocations[0].name
        if alloc.kind == "ExternalInput":
            if name != part_name:
                in_names.append(name)
        elif alloc.kind == "ExternalOutput":
            out_names.append(name)
            out_avals.append(
                jax.core.ShapedArray(tuple(alloc.tensor_shape), mybir.dt.np(alloc.dtype))
            )
    n_params = len(in_names)
    all_in = in_names + out_names
    if part_name is not None:
        all_in = all_in + [part_name]

    def _body(*args):
        operands = list(args)
        if part_name is not None:
            operands.append(bass2jax.partition_id_tensor())
        return tuple(
            p.bind(
                *operands,
                out_avals=tuple(out_avals),
                in_names=tuple(all_in),
                out_names=tuple(out_names),
                lowering_input_output_aliases=(),
                sim_require_finite=True,
                sim_require_nnan=True,
                nc=nc,
            )
        )

    devices = jax.devices()[:NCORES]
    mesh = Mesh(np.asarray(devices), ("core",))
    nin = n_params + len(out_names)
    sharded = jax.jit(
        shard_map(
            _body,
            mesh=mesh,
            in_specs=(PartitionSpec("core"),) * nin,
            out_specs=(PartitionSpec("core"),) * len(out_names),
            check_rep=False,
        ),
        donate_argnums=tuple(range(n_params, nin)),
        keep_unused=True,
    )
    return sharded, in_names, out_names, out_avals, mesh


def get_runner():
    global _NC, _RUNNER
    if _RUNNER is None:
        if _NC is None:
            _NC = build_nc()
        _RUNNER = _make_runner(_NC)
    return _RUNNER


def make_concat_inputs(inter_times, h_ti, c_ti, cbar, W, b):
    """Global (8*shape[0], ...) arrays keyed by dram tensor name."""
    BF = np.float16
    inter_times = np.asarray(inter_times, dtype=np.float32)
    wt, bia = prep_weights(np.asarray(W, np.float32), np.asarray(b, np.float32))
    dts = np.ascontiguousarray(
        inter_times.reshape(NCORES, NBLK, P).transpose(0, 2, 1)
    ).reshape(NCORES * P, NBLK)
    # hT: [core, k-chunk, feature, row] -> concat [(core chunk), P, BL]
    h_bf = np.asarray(h_ti).astype(BF)
    hT = np.ascontiguousarray(
        h_bf.reshape(NCORES, BL, 2, P).transpose(0, 2, 3, 1)
    ).reshape(NCORES * 2, P, BL)
    # c/cbar: partition-major [core, row%128, block, col] -> [(core P), NBLK*H]
    def pmaj(x):
        xb = np.asarray(x).astype(BF)
        return np.ascontiguousarray(
            xb.reshape(NCORES, NBLK, P, H).transpose(0, 2, 1, 3)
        ).reshape(NCORES * P, NBLK * H)

    return {
        "hT": hT,
        "c": pmaj(c_ti),
        "cbar": pmaj(cbar),
        "dts": dts,
        "wt": np.ascontiguousarray(np.broadcast_to(wt, (NCORES,) + wt.shape)).reshape(
            NCORES * wt.shape[0], *wt.shape[1:]
        ),
        "bias": np.ascontiguousarray(
            np.broadcast_to(bia, (NCORES,) + bia.shape)
        ).reshape(NCORES * bia.shape[0], *bia.shape[1:]),
    }


def device_zeros(out_avals, mesh):
    import jax.numpy as jnp
    from jax.sharding import NamedSharding, PartitionSpec

    sh = NamedSharding(mesh, PartitionSpec("core"))
    return [
        jnp.zeros((NCORES * a.shape[0], *a.shape[1:]), a.dtype, device=sh)
        for a in out_avals
    ]


def kernel(inter_times, h_ti, c_ti, cbar, W, b):
    sharded, in_names, out_names, out_avals, mesh = get_runner()
    cat = make_concat_inputs(inter_times, h_ti, c_ti, cbar, W, b)
    zeros = device_zeros(out_avals, mesh)
    out_arrs = sharded(*[cat[n] for n in in_names], *zeros)
    by_name = {n: np.asarray(a) for n, a in zip(out_names, out_arrs)}
    return tuple(
        by_name[n].astype(np.float32) for n in ["og", "hn", "cn", "cbn", "dec"]
    )


# revision 32
# speedup vs baseline: 1.3588x; 1.0950x over previous
"""Trainium2 Bass kernel for NeuralCTLSTM cell (B=65536, H=256, 7 gates).

Data-parallel over 8 NeuronCores (8192 batch rows each). Per core, per
128-row block:
  gates = h @ Wp^T + bp  (7 gates in one PSUM tile [128,1792], bf16 matmuls,
                          K=256 in 2 chunks + a K=1 ones-row for the bias)
  gate columns: [f, o, fbar, i, ibar, dneg, z] where the dneg slot holds the
  NEGATED decay-gate weights, so sigmoid drains 6 contiguous gates at once:
    sigma(-g_d) -> lns = ln(sigma(-g_d)) = -softplus(g_d) = -decay
    decay = -lns ; e = exp(-dt*decay) = exp(dt*lns)
  h^T needed by the PE comes straight from DRAM via DMA-transpose (bf16).
  c_after = cbar + (c-cbar)*e ; outputs og, hn, cn, cbn, dec (all bf16,
  host converts to fp32).

ACT table phasing: supergroups of 16 blocks; sigmoid/tanh (sigmoid set) for
the supergroup, then ln/exp (natural_log_exp set). tanh(c_after) of
supergroup g runs at the start of the sigmoid phase of g+1. Zero-valued
[P,1] tokens (passed as activation bias) enforce the phase edges.
"""

import os
import sys

sys.path.insert(0, "/opt/trn_rl_repo")

from contextlib import ExitStack

import numpy as np

NCORES = 8
B, H, G = 65536, 256, 7
P = 128
BL = B // NCORES           # rows per core
NBLK = BL // P             # 64 row-blocks per core
ST = 4                     # blocks per group (DMA/DVE batch)
NGRP = NBLK // ST          # 16 groups per core
GPS = 4                    # groups per supergroup
NSG = NGRP // GPS          # 4 supergroups
GH = G * H                 # 1792
SIGW = 6 * H               # 1536: f,o,fbar,i,ibar,dneg drained by one sigmoid

# gate column slots -> reference gate index (reference order:
# 0=input 1=forget 2=output 3=ibar 4=fbar 5=z 6=decay). Slot 5 (dneg) is
# negated on the host.
GATE_PERM = [1, 2, 4, 0, 3, 6, 5]

_NC = None


def build_nc():
    from concourse import bacc, mybir
    from concourse.tile import TileContext

    F32 = mybir.dt.float32
    F16 = mybir.dt.float16
    AF = mybir.ActivationFunctionType

    nc = bacc.Bacc("TRN2", target_bir_lowering=False, debug=False)

    # hT: host-pre-transposed h, [k-chunk, feature, batch-row] per core.
    # c/cbar: partition-major [row%128, block*H + col] so per-partition DMA
    # segments are contiguous (2 KB per block instead of 512 B lines).
    hT_d = nc.dram_tensor("hT", [2, P, BL], F16, kind="ExternalInput")
    c_d = nc.dram_tensor("c", [P, NBLK * H], F16, kind="ExternalInput")
    cb_d = nc.dram_tensor("cbar", [P, NBLK * H], F16, kind="ExternalInput")
    dt_d = nc.dram_tensor("dts", [P, NBLK], F32, kind="ExternalInput")
    wt_d = nc.dram_tensor("wt", [2, P, GH], F16, kind="ExternalInput")
    bia_d = nc.dram_tensor("bias", [4, GH], F16, kind="ExternalInput")

    og_d = nc.dram_tensor("og", [BL, H], F16, kind="ExternalOutput")
    hn_d = nc.dram_tensor("hn", [BL, H], F16, kind="ExternalOutput")
    cn_d = nc.dram_tensor("cn", [BL, H], F16, kind="ExternalOutput")
    cbn_d = nc.dram_tensor("cbn", [BL, H], F16, kind="ExternalOutput")
    dec_d = nc.dram_tensor("dec", [BL, H], F16, kind="ExternalOutput")

    og_r = og_d.rearrange("(n p) d -> n p d", p=P)
    hn_r = hn_d.rearrange("(n p) d -> n p d", p=P)
    cn_r = cn_d.rearrange("(n p) d -> n p d", p=P)
    cbn_r = cbn_d.rearrange("(n p) d -> n p d", p=P)
    dec_r = dec_d.rearrange("(n p) d -> n p d", p=P)

    def dsl(r, b0, n):  # dram slice of n row-blocks as [P, n, H]
        return r[b0 : b0 + n].rearrange("n p d -> p n d")

    MU = mybir.AluOpType.mult

    with TileContext(nc) as tc, ExitStack() as ctx:
        pool = lambda name, bufs, **kw: ctx.enter_context(
            tc.tile_pool(name=name, bufs=bufs, **kw)
        )
        const = pool("const", 1)
        hTp = pool("hTp", 2)
        cp = pool("cp", 2)
        cbp = pool("cbp", 2)
        Sp = pool("Sp", 6)
        ztp = pool("ztp", 2)
        lnsp = pool("lnsp", 4)
        einp = pool("einp", 2)
        ep = pool("ep", 4)
        cap = pool("cap", 6)
        tcap = pool("tcap", 2)
        tmpp = pool("tmpp", 2)
        hnp = pool("hnp", 2)
        cnp = pool("cnp", 2)
        cbnp = pool("cbnp", 2)
        ndecp = pool("ndecp", 2)
        tokp = pool("tokp", 4)
        pgp = pool("pgp", 2, space="PSUM")

        # bias/ones replicated at partitions 0/32/64/96: the four K=1 bias
        # matmuls go to distinct PE row-groups via tile_position and run
        # concurrently (~215ns instead of ~747ns per block).
        bia_sb = const.tile([P, GH], F16)
        for i in range(4):
            nc.sync.dma_start(bia_sb[32 * i : 32 * i + 1, :], bia_d[i : i + 1, :])
        wt_sb = const.tile([P, 2, GH], F16)
        nc.sync.dma_start(wt_sb[:], wt_d.rearrange("c k g -> k c g"))
        dts_sb = const.tile([P, NBLK], F32)
        nc.sync.dma_start(dts_sb[:], dt_d[:, :])
        ones_sb = const.tile([P, P], F16)
        nc.vector.memset(ones_sb[:], 1.0)

        BANKS = [(0, 512), (512, 1024), (1024, 1536), (1536, 1792)]

        tokA = None        # gates SIG-phase ops of SG g after NLE ops of g-1
        prev = []          # [(S, ca, grp), ...] of previous supergroup

        SGB = GPS * ST  # blocks per supergroup (16)

        def issue_sg_inputs(g):
            hT = hTp.tile([P, 2, SGB * P], F16)
            nc.sync.dma_start(
                hT[:],
                hT_d[:, :, g * SGB * P : (g + 1) * SGB * P].rearrange(
                    "c k r -> k c r"
                ),
            )
            c_sg = cp.tile([P, SGB, H], F16)
            cb_sg = cbp.tile([P, SGB, H], F16)
            nc.sync.dma_start(
                c_sg[:],
                c_d[:, g * SGB * H : (g + 1) * SGB * H].rearrange(
                    "k (n d) -> k n d", d=H
                ),
            )
            nc.sync.dma_start(
                cb_sg[:],
                cb_d[:, g * SGB * H : (g + 1) * SGB * H].rearrange(
                    "k (n d) -> k n d", d=H
                ),
            )
            return hT, c_sg, cb_sg

        cur_in = issue_sg_inputs(0)
        for g in range(NSG):
            # ---------------- SIG phase: matmuls + sigmoid/tanh drains ------
            hT, c_sg, cb_sg = cur_in
            nxt_in = issue_sg_inputs(g + 1) if g + 1 < NSG else None
            sg_S = []
            tca_last = None
            for t in range(GPS):
                S = Sp.tile([P, ST, GH], F16)
                for j in range(ST):
                    jj = t * ST + j
                    Gp = pgp.tile([P, 2048], mybir.dt.float32)
                    for i, (lo, hi) in enumerate(BANKS):
                        bp32 = 32 * i
                        nc.tensor.matmul(
                            Gp[:, lo:hi],
                            ones_sb[bp32 : bp32 + 1, :],
                            bia_sb[bp32 : bp32 + 1, lo:hi],
                            start=True, stop=False,
                            tile_position=(bp32, 0),
                        )
                    for ck in range(2):
                        lhsT = hT[:, ck, jj * P : (jj + 1) * P]
                        for lo, hi in BANKS:
                            nc.tensor.matmul(
                                Gp[:, lo:hi], lhsT, wt_sb[:, ck, lo:hi],
                                start=False, stop=(ck == 1),
                            )
                    if tokA is not None:
                        nc.scalar.activation(
                            S[:, j, :], Gp[:, 0:GH], AF.Sigmoid, bias=tokA[:]
                        )
                    else:
                        nc.scalar.activation(S[:, j, :], Gp[:, 0:GH], AF.Sigmoid)
                sg_S.append(S)
                nc.sync.dma_start(
                    dsl(og_r, (g * GPS + t) * ST, ST), S[:, :, H : 2 * H]
                )
                # previous supergroup, same group index: tanh(c_after) + hn
                if prev:
                    pS, pca, pgrp = prev[t]
                    tca = tcap.tile([P, ST, H], F16)
                    if tokA is not None:
                        nc.scalar.activation(tca[:], pca[:], AF.Tanh, bias=tokA[:])
                    else:
                        nc.scalar.activation(tca[:], pca[:], AF.Tanh)
                    hn = hnp.tile([P, ST, H], F16)
                    nc.vector.tensor_mul(hn[:], pS[:, :, H : 2 * H], tca[:])
                    nc.sync.dma_start(dsl(hn_r, pgrp * ST, ST), hn[:])
                    tca_last = tca

            # tokB: all NLE-set ACT ops of this SG wait on the SIG-set ops
            tokB = tokp.tile([P, 1], mybir.dt.float32)
            zsrc = sg_S[-1][:, ST - 1, 6 * H : 6 * H + 1]
            if tca_last is not None:
                nc.vector.scalar_tensor_tensor(
                    tokB[:], zsrc, 0.0, tca_last[:, ST - 1, 0:1], MU, MU,
                )
            else:
                nc.vector.tensor_scalar_mul(tokB[:], zsrc, 0.0)

            # ---------------- NLE phase: ln/exp + elementwise + outputs -----
            # All Ln ops first, then all Exp ops: one natural_log table load
            # and one exp_and_others load per supergroup instead of 2 each.
            sg_lns = []
            for t in range(GPS):
                lns = lnsp.tile([P, ST, H], F16)
                nc.scalar.activation(
                    lns[:], sg_S[t][:, :, 5 * H : 6 * H], AF.Ln, bias=tokB[:]
                )
                sg_lns.append(lns)
            sg_e = []
            for t in range(GPS):
                b0 = (g * GPS + t) * ST
                lns = sg_lns[t]
                ndec = ndecp.tile([P, ST, H], F16)
                nc.vector.tensor_scalar_mul(ndec[:], lns[:], -1.0)
                nc.sync.dma_start(dsl(dec_r, b0, ST), ndec[:])
                ein = einp.tile([P, ST, H], F16)
                for j in range(ST):
                    blk = b0 + j
                    nc.vector.tensor_scalar_mul(
                        ein[:, j, :], lns[:, j, :], dts_sb[:, blk : blk + 1]
                    )
                e = ep.tile([P, ST, H], F16)
                nc.scalar.activation(e[:], ein[:], AF.Exp, bias=tokB[:])
                sg_e.append(e)
            # tokA must fire as soon as the last Exp lands — BEFORE the DVE
            # chains below — or the next supergroup's sigmoids (gated on it)
            # sit behind ~27us of chain work in the in-order DVE queue.
            tokA = tokp.tile([P, 1], mybir.dt.float32)
            nc.vector.tensor_scalar_mul(tokA[:], sg_e[-1][:, ST - 1, 0:1], 0.0)
            prev_new = []
            for t in range(GPS):
                grp = g * GPS + t
                b0 = grp * ST
                S, e = sg_S[t], sg_e[t]
                zt = ztp.tile([P, ST, H], F16)
                nc.vector.tensor_scalar(
                    zt[:], S[:, :, 6 * H : 7 * H], 2.0, -1.0,
                    mybir.AluOpType.mult, mybir.AluOpType.add,
                )
                c_t = c_sg[:, t * ST : (t + 1) * ST, :]
                cb_t = cb_sg[:, t * ST : (t + 1) * ST, :]
                # elementwise chain (DVE, bf16 2x); two reused scratch tiles
                tA = tmpp.tile([P, ST, H], F16)
                tB = tmpp.tile([P, ST, H], F16)
                nc.vector.tensor_sub(tA[:], c_t, cb_t)          # d = c - cb
                nc.vector.tensor_mul(tB[:], tA[:], e[:])        # m = d * e
                ca = cap.tile([P, ST, H], F16)
                nc.vector.tensor_add(ca[:], tB[:], cb_t)        # ca = m + cb
                nc.vector.tensor_mul(tA[:], S[:, :, 3 * H : 4 * H], zt[:])  # iz
                nc.vector.tensor_mul(tB[:], S[:, :, 0:H], ca[:])           # f*ca
                cn = cnp.tile([P, ST, H], F16)
                nc.vector.tensor_add(cn[:], tB[:], tA[:])
                nc.sync.dma_start(dsl(cn_r, b0, ST), cn[:])
                nc.vector.tensor_mul(tA[:], S[:, :, 2 * H : 3 * H], cb_t)  # fbar*cb
                nc.vector.tensor_mul(tB[:], S[:, :, 4 * H : 5 * H], zt[:])  # ibar*z
                cbn = cbnp.tile([P, ST, H], F16)
                nc.vector.tensor_add(cbn[:], tA[:], tB[:])
                nc.sync.dma_start(dsl(cbn_r, b0, ST), cbn[:])
                prev_new.append((S, ca, grp))

            prev = prev_new
            cur_in = nxt_in

        # epilogue: hn for the final supergroup
        for t in range(GPS):
            pS, pca, pgrp = prev[t]
            tca = tcap.tile([P, ST, H], F16)
            nc.scalar.activation(tca[:], pca[:], AF.Tanh, bias=tokA[:])
            hn = hnp.tile([P, ST, H], F16)
            nc.vector.tensor_mul(hn[:], pS[:, :, H : 2 * H], tca[:])
            nc.sync.dma_start(dsl(hn_r, pgrp * ST, ST), hn[:])

    nc.compile()
    return nc


def prep_weights(W, b):
    """W [7,256,256] fp32, b [7,256] fp32 -> wt [2,128,1792] fp16, bias [1,1792] fp16.

    Slot 5 (decay gate) negated so sigma(-g_d) comes out of the big sigmoid;
    slot 6 (z gate) doubled so tanh(g_z) = 2*sigma(2 g_z) - 1.
    """
    Wp = np.ascontiguousarray(W[GATE_PERM]).astype(np.float32).copy()
    bp = np.ascontiguousarray(b[GATE_PERM]).astype(np.float32).copy()
    Wp[5] = -Wp[5]  # dneg slot: sigma(-g_d)
    bp[5] = -bp[5]
    Wp[6] = 2.0 * Wp[6]  # z slot: sigma(2 g_z)
    bp[6] = 2.0 * bp[6]
    # wt[ck, k, g*H+o] = Wp[g, o, ck*128+k]
    wt = np.transpose(Wp, (2, 0, 1)).reshape(H, GH)
    wt = np.ascontiguousarray(wt.reshape(2, P, GH)).astype(np.float16)
    bia = np.ascontiguousarray(
        np.broadcast_to(bp.reshape(1, GH), (4, GH))
    ).astype(np.float16)
    return wt, bia


_RUNNER = None


def _make_runner(nc):
    """Cached shard_map-jitted executor for nc across 8 cores."""
    import jax
    from jax.experimental.shard_map import shard_map
    from jax.sharding import Mesh, PartitionSpec

    from concourse import bass2jax, mybir

    bass2jax.install_neuronx_cc_hook()
    p = bass2jax._bass_exec_p

    part_name = nc.partition_id_tensor.name if nc.partition_id_tensor else None
    in_names, out_names, out_avals = [], [], []
    for alloc in nc.m.functions[0].allocations:
        if not isinstance(alloc, mybir.MemoryLocationSet):
            continue
        name = alloc.memorylocations[0].name
        if alloc.kind == "ExternalInput":
            if name != part_name:
                in_names.append(name)
        elif alloc.kind == "ExternalOutput":
            out_names.append(name)
            out_avals.append(
                jax.core.ShapedArray(tuple(alloc.tensor_shape), mybir.dt.np(alloc.dtype))
            )
    n_params = len(in_names)
    all_in = in_names + out_names
    if part_name is not None:
        all_in = all_in + [part_name]

    def _body(*args):
        operands = list(args)
        if part_name is not None:
            operands.append(bass2jax.partition_id_tensor())
        return tuple(
            p.bind(
                *operands,
                out_avals=tuple(out_avals),
                in_names=tuple(all_in),
                out_names=tuple(out_names),
                lowering_input_output_aliases=(),
                sim_require_finite=True,
                sim_require_nnan=True,
                nc=nc,
            )
        )

    devices = jax.devices()[:NCORES]
    mesh = Mesh(np.asarray(devices), ("core",))
    nin = n_params + len(out_names)
    sharded = jax.jit(
        shard_map(
            _body,
            mesh=mesh,
            in_specs=(PartitionSpec("core"),) * nin,
            out_specs=(PartitionSpec("core"),) * len(out_names),
            check_rep=False,
        ),
        donate_argnums=tuple(range(n_params, nin)),
        keep_unused=True,
    )
    return sharded, in_names, out_names, out_avals, mesh


def get_runner():
    global _NC, _RUNNER
    if _RUNNER is None:
        if _NC is None:
            _NC = build_nc()
        _RUNNER = _make_runner(_NC)
    return _RUNNER


def make_concat_inputs(inter_times, h_ti, c_ti, cbar, W, b):
    """Global (8*shape[0], ...) arrays keyed by dram tensor name."""
    BF = np.float16
    inter_times = np.asarray(inter_times, dtype=np.float32)
    wt, bia = prep_weights(np.asarray(W, np.float32), np.asarray(b, np.float32))
    dts = np.ascontiguousarray(
        inter_times.reshape(NCORES, NBLK, P).transpose(0, 2, 1)
    ).reshape(NCORES * P, NBLK)
    # hT: [core, k-chunk, feature, row] -> concat [(core chunk), P, BL]
    h_bf = np.asarray(h_ti).astype(BF)
    hT = np.ascontiguousarray(
        h_bf.reshape(NCORES, BL, 2, P).transpose(0, 2, 3, 1)
    ).reshape(NCORES * 2, P, BL)
    # c/cbar: partition-major [core, row%128, block, col] -> [(core P), NBLK*H]
    def pmaj(x):
        xb = np.asarray(x).astype(BF)
        return np.ascontiguousarray(
            xb.reshape(NCORES, NBLK, P, H).transpose(0, 2, 1, 3)
        ).reshape(NCORES * P, NBLK * H)

    return {
        "hT": hT,
        "c": pmaj(c_ti),
        "cbar": pmaj(cbar),
        "dts": dts,
        "wt": np.ascontiguousarray(np.broadcast_to(wt, (NCORES,) + wt.shape)).reshape(
            NCORES * wt.shape[0], *wt.shape[1:]
        ),
        "bias": np.ascontiguousarray(
            np.broadcast_to(bia, (NCORES,) + bia.shape)
        ).reshape(NCORES * bia.shape[0], *bia.shape[1:]),
    }


def device_zeros(out_avals, mesh):
    import jax.numpy as jnp
    from jax.sharding import NamedSharding, PartitionSpec

    sh = NamedSharding(mesh, PartitionSpec("core"))
    return [
        jnp.zeros((NCORES * a.shape[0], *a.shape[1:]), a.dtype, device=sh)
        for a in out_avals
    ]


def kernel(inter_times, h_ti, c_ti, cbar, W, b):
    sharded, in_names, out_names, out_avals, mesh = get_runner()
    cat = make_concat_inputs(inter_times, h_ti, c_ti, cbar, W, b)
    zeros = device_zeros(out_avals, mesh)
    out_arrs = sharded(*[cat[n] for n in in_names], *zeros)
    by_name = {n: np.asarray(a) for n, a in zip(out_names, out_arrs)}
    return tuple(
        by_name[n].astype(np.float32) for n in ["og", "hn", "cn", "cbn", "dec"]
    )
